# revision 25
# baseline (speedup 1.0000x reference)
"""TRN2 Bass kernel for nn_AttentionalDynamicsUpdate (dense transformer block).

Math per sequence (K=64 tokens, D=512, E=2048):
    q = h @ W_q.T; k = [h @ W_hk.T | z @ W_zk.T]; v = [h @ W_hv.T | z @ W_zv.T]
    logits = k @ q.T / sqrt(D); p = softmax(logits, axis=q)
    out = layernorm((p @ v) @ W_out.T)

Kernel reorderings (exact up to fp rounding):
  * (p @ v) @ W_out.T == p @ (v @ W_out.T)  -- turns the 2048-wide attn
    output into a 512-wide "u" computed as one dense batched matmul.
  * layernorm is scale-invariant per row, so the softmax 1/sum(exp)
    normalization is skipped entirely (absorbed by the layernorm).

Data-parallel over the N=256 sequences across 8 cores (32 seqs / core).
Matmul operands are float32r (TF32-like; 1 cycle/row at free-dim >= 256),
accumulation fp32 in PSUM, softmax/layernorm in fp32.
Host pre-transposes h/z/weights so every operand arrives feature-major.
"""

import math

import numpy as np

import concourse.bacc as bacc
import concourse.bass as bass
import concourse.mybir as mybir
import concourse.tile as tile
from concourse.bass_utils import run_bass_kernel_spmd

N_CORES = 8
N_SEQ, SEQ_K, D = 256, 64, 512
E = 2048  # concat feature width (also query width)
TPC = (N_SEQ // N_CORES) * SEQ_K  # tokens per core = 2048
TC = 512  # tokens per pipeline chunk (8 seqs, 4 pairs)
NCH = TPC // TC  # 4 chunks
EC = E // 128  # 16 e-chunks
DC = D // 128  # 4 d-chunks
NPAIR = TC // 128  # 4 seq-pairs per chunk
SCALE = 1.0 / math.sqrt(D)
LN_EPS = 1e-5

F32 = mybir.dt.float32
F32R = mybir.dt.float32r
BF16 = mybir.dt.bfloat16
AX = mybir.AxisListType.X
OP = mybir.AluOpType
AF = mybir.ActivationFunctionType

# wt feature-column layout: [hk 0:1024 | hv 1024:2048 | q 2048:4096 |
#                            zk 4096:5120 | zv 5120:6144]
W_COLS = 6144


def _qkv_src(c):
    """(q_col0, k_src, k_col0, v_src, v_col0) for e-chunk c; src 0=hT 1=zT."""
    q0 = 2048 + 128 * c
    if c < 8:
        return q0, 0, 128 * c, 0, 1024 + 128 * c
    return q0, 1, 4096 + 128 * (c - 8), 1, 5120 + 128 * (c - 8)


def build(fast_ln: bool):
    nc = bacc.Bacc("TRN2", target_bir_lowering=False)

    hT = nc.dram_tensor("hT", [DC, 128, TPC], F32R, kind="ExternalInput")
    zT = nc.dram_tensor("zT", [DC, 128, TPC], F32R, kind="ExternalInput")
    wt = nc.dram_tensor("wt", [DC, 128, W_COLS], F32R, kind="ExternalInput")
    wout = nc.dram_tensor("wout", [EC, 128, D], F32R, kind="ExternalInput")
    gb = nc.dram_tensor("gb", [2, 128, D], F32, kind="ExternalInput")
    ident_dram = nc.inline_tensor(np.eye(128, dtype=np.float32), name="ident128")
    out = nc.dram_tensor("out", [TPC, D], F32, kind="ExternalOutput")

    with tile.TileContext(nc) as tc:
        with (
            tc.tile_pool(name="wpool", bufs=1) as wpool,
            tc.tile_pool(name="xpool", bufs=2) as xpool,
            tc.tile_pool(name="wopool", bufs=3) as wopool,
            tc.tile_pool(name="qkv", bufs=4) as qkv,
            tc.tile_pool(name="attn", bufs=2) as attn,
            tc.tile_pool(name="vecs", bufs=4) as vecs,
            tc.tile_pool(name="psproj", bufs=3, space="PSUM") as psproj,
            tc.tile_pool(name="psu", bufs=4, space="PSUM") as psu,
            tc.tile_pool(name="pslg", bufs=1, space="PSUM") as pslg,
        ):
            # chunk-0 activations first: the first projection matmuls need
            # these plus only the first weight block
            x_tiles = {}
            for tch0 in range(2):
                hT_sb0 = xpool.tile(
                    [128, DC, TC], F32R, name="hT_sb", tag="hT_sb"
                )
                zT_sb0 = xpool.tile(
                    [128, DC, TC], F32R, name="zT_sb", tag="zT_sb"
                )
                x_tiles[tch0] = (hT_sb0, zT_sb0)
            wt_sb = wpool.tile([128, DC, W_COLS], F32R)
            # interleave chunk-0 h with the first-needed weight block so the
            # first projection matmul's inputs land within a few us; z-side
            # loads are deferred (first needed at e-chunk 8)
            for d in range(DC):
                nc.sync.dma_start(x_tiles[0][0][:, d, :], hT[d, :, 0:TC])
                nc.gpsimd.dma_start(wt_sb[:, d, 2048:3072], wt[d, :, 2048:3072])
            for c0, c1 in ((0, 1024), (1024, 2048)):  # hk, hv
                for d in range(DC):
                    eng = nc.scalar if d % 2 else nc.sync
                    eng.dma_start(wt_sb[:, d, c0:c1], wt[d, :, c0:c1])
            for d in range(DC):
                nc.gpsimd.dma_start(x_tiles[0][1][:, d, :], zT[d, :, 0:TC])
            for c0, c1 in ((4096, 5120), (5120, 6144), (3072, 4096)):
                for d in range(DC):
                    eng = nc.gpsimd if d % 2 else nc.scalar
                    eng.dma_start(wt_sb[:, d, c0:c1], wt[d, :, c0:c1])
            ident = wpool.tile([128, 128], F32)
            nc.sync.dma_start(ident[:], ident_dram[:])
            eps_t = wpool.tile([128, 1], F32)
            nc.vector.memset(eps_t[:], LN_EPS)
            if not fast_ln:
                gtile = wpool.tile([128, D], F32)
                btile = wpool.tile([128, D], F32)
                nc.sync.dma_start(gtile[:], gb[0])
                nc.sync.dma_start(btile[:], gb[1])

            for tch in range(NCH):
                t0 = tch * TC
                if tch == 0:
                    hT_sb, zT_sb = x_tiles[0]
                else:
                    if tch == 1:
                        hT_sb, zT_sb = x_tiles[1]
                    else:
                        hT_sb = xpool.tile(
                            [128, DC, TC], F32R, name="hT_sb", tag="hT_sb"
                        )
                        zT_sb = xpool.tile(
                            [128, DC, TC], F32R, name="zT_sb", tag="zT_sb"
                        )
                    for d in range(DC):
                        nc.sync.dma_start(hT_sb[:, d, :], hT[d, :, t0 : t0 + TC])
                        nc.gpsimd.dma_start(zT_sb[:, d, :], zT[d, :, t0 : t0 + TC])
                srcs = (hT_sb, zT_sb)

                lg_acc = attn.tile(
                    [128, NPAIR, 128], F32, name="lg_acc", tag="lgacc"
                )
                u_ps = [
                    psu.tile([128, D], F32, name=f"u_ps{g}", tag="ub")
                    for g in range(NPAIR)
                ]

                for c in range(EC):
                    q0, ksrc, k0, vsrc, v0 = _qkv_src(c)
                    wo_sb = wopool.tile([128, D], F32R, name="wo_sb", tag="wo")
                    nc.gpsimd.dma_start(wo_sb[:], wout[c])

                    q_ps = psproj.tile([128, TC], F32, name="q_ps", tag="proj")
                    k_ps = psproj.tile([128, TC], F32, name="k_ps", tag="proj")
                    v_ps = psproj.tile([128, TC], F32, name="v_ps", tag="proj")
                    for ps, src, col0 in (
                        (q_ps, hT_sb, q0),
                        (k_ps, srcs[ksrc], k0),
                        (v_ps, srcs[vsrc], v0),
                    ):
                        for d in range(DC):
                            nc.tensor.matmul(
                                ps[:],
                                wt_sb[:, d, col0 : col0 + 128],
                                src[:, d, :],
                                start=(d == 0),
                                stop=(d == DC - 1),
                            )
                    q_sb = qkv.tile([128, TC], BF16, name="q_sb", tag="q_sb")
                    k_sb = qkv.tile([128, TC], BF16, name="k_sb", tag="k_sb")
                    v_sb = qkv.tile([128, TC], F32R, name="v_sb", tag="v_sb")
                    nc.scalar.copy(q_sb[:], q_ps[:])
                    nc.vector.tensor_copy(k_sb[:], k_ps[:])
                    nc.scalar.copy(v_sb[:], v_ps[:])

                    first = c == 0
                    # full [kA|kB] x [qA|qB] blocks; the diagonal 64x64
                    # sub-blocks are the two sequences' logits (cross terms
                    # are discarded). PSUM accumulation across e-chunks is
                    # unsafe (pairs share a bank and start=True clears the
                    # whole bank's has_written bits), so the partials land
                    # in one bank and accumulate in SBUF with one DVE add.
                    lgp = pslg.tile([128, NPAIR, 128], F32, name="lgp", tag="lgp")
                    for p in range(NPAIR):
                        pb = p * 128
                        nc.tensor.matmul(
                            lgp[:, p, :],
                            k_sb[:, pb : pb + 128],
                            q_sb[:, pb : pb + 128],
                        )
                        nc.tensor.matmul(
                            u_ps[p][:],
                            v_sb[:, pb : pb + 128],
                            wo_sb[:],
                            start=first,
                            stop=(c == EC - 1),
                        )
                    if first:
                        nc.vector.tensor_copy(lg_acc[:], lgp[:])
                    else:
                        nc.vector.tensor_add(lg_acc[:], lg_acc[:], lgp[:])

                # phase A: softmax + probs^T + u re-layout for all pairs
                # (groups ACT functions to avoid act-table thrash)
                pt_sbs, u_sbs = [], []
                for p in range(NPAIR):
                    mx = vecs.tile([128, 1], F32, name="mx", tag="mx")
                    mneg = vecs.tile([128, 1], F32, name="mneg", tag="mneg")
                    nc.vector.reduce_max(mx[0:64], lg_acc[0:64, p, 0:64], axis=AX)
                    nc.vector.reduce_max(
                        mx[64:128], lg_acc[64:128, p, 64:128], axis=AX
                    )
                    nc.vector.tensor_scalar_mul(mneg[:], mx[:], -SCALE)
                    probs = attn.tile([128, 64], F32, name="probs", tag="probs")
                    nc.scalar.activation(
                        probs[0:64, :],
                        lg_acc[0:64, p, 0:64],
                        AF.Exp,
                        bias=mneg[0:64],
                        scale=SCALE,
                    )
                    nc.scalar.activation(
                        probs[64:128, :],
                        lg_acc[64:128, p, 64:128],
                        AF.Exp,
                        bias=mneg[64:128],
                        scale=SCALE,
                    )
                    # probs^T: [64 q, 128 (kA|kB)]; transpose out must sit at
                    # PSUM partition 0
                    pt_ps = pslg.tile([64, 128], F32, name="pt_ps", tag="lgp")
                    nc.tensor.transpose(pt_ps[:], probs[:], ident[:])
                    pt_sb = attn.tile(
                        [64, 128], F32R, name=f"pt_sb{p}", tag=f"ptsb{p}", bufs=1
                    )
                    nc.vector.tensor_copy(pt_sb[:], pt_ps[:])
                    pt_sbs.append(pt_sb)

                    # u re-layout: both seq halves to partition base 0 (DMA
                    # shifts partitions; DVE cannot; DMA cannot read PSUM)
                    u_st = attn.tile([128, D], F32R, name="u_st", tag="ust")
                    nc.scalar.copy(u_st[:], u_ps[p][:])
                    u_sb = attn.tile(
                        [64, 2, D], F32R, name=f"u_sb{p}", tag=f"usb{p}", bufs=1
                    )
                    nc.vector.tensor_copy(u_sb[:, 0, :], u_st[0:64, :])
                    nc.gpsimd.dma_start(u_sb[:, 1, :], u_st[64:128, :])
                    u_sbs.append(u_sb)

                # phase B: out matmuls + layernorm + store
                for p in range(NPAIR):
                    pt_sb, u_sb = pt_sbs[p], u_sbs[p]
                    o_ps = [
                        psu.tile([64, D], F32, name=f"o_ps{si}", tag="ub")
                        for si in range(2)
                    ]
                    nc.tensor.matmul(o_ps[0][:], pt_sb[:, 0:64], u_sb[:, 0, :])
                    nc.tensor.matmul(o_ps[1][:], pt_sb[:, 64:128], u_sb[:, 1, :])

                    # layernorm over D; softmax 1/sum already absorbed here
                    for si in range(2):
                        oraw = attn.tile([64, D], F32, name="oraw", tag="oraw")
                        nc.vector.tensor_copy(oraw[:], o_ps[si][:])
                        ops = oraw
                        sm = vecs.tile([64, 1], F32, name="sm", tag="sm")
                        ssq = vecs.tile([64, 1], F32, name="ssq", tag="ssq")
                        mu = vecs.tile([64, 1], F32, name="mu", tag="mu")
                        mu2 = vecs.tile([64, 1], F32, name="mu2", tag="mu2")
                        var = vecs.tile([64, 1], F32, name="var", tag="var")
                        sd = vecs.tile([64, 1], F32, name="sd", tag="sd")
                        rstd = vecs.tile([64, 1], F32, name="rstd", tag="rstd")
                        c1 = vecs.tile([64, 1], F32, name="c1", tag="c1")
                        scr = attn.tile([64, D], F32, name="scr", tag="scr")
                        nc.vector.reduce_sum(sm[:], ops[:], axis=AX)
                        nc.scalar.activation(
                            scr[:], ops[:], AF.Square, accum_out=ssq[:]
                        )
                        nc.vector.tensor_scalar_mul(mu[:], sm[:], 1.0 / D)
                        nc.vector.tensor_mul(mu2[:], mu[:], mu[:])
                        nc.vector.tensor_scalar_mul(var[:], ssq[:], 1.0 / D)
                        nc.vector.tensor_sub(var[:], var[:], mu2[:])
                        nc.scalar.activation(sd[:], var[:], AF.Sqrt, bias=eps_t[0:64])
                        nc.vector.reciprocal(rstd[:], sd[:])
                        nc.vector.tensor_scalar(
                            c1[:], mu[:], rstd[:], -1.0, op0=OP.mult, op1=OP.mult
                        )
                        o_sb = attn.tile([64, D], F32, name="o_sb", tag="osb")
                        nc.vector.tensor_scalar(
                            o_sb[:], ops[:], rstd[:], c1[:], op0=OP.mult, op1=OP.add
                        )
                        if not fast_ln:
                            nc.vector.tensor_mul(o_sb[:], o_sb[:], gtile[0:64, :])
                            nc.vector.tensor_add(o_sb[:], o_sb[:], btile[0:64, :])
                        r0 = t0 + p * 128 + si * 64
                        nc.scalar.dma_start(out[r0 : r0 + 64, :], o_sb[:])

    nc.compile()
    return nc


_NC_CACHE = {}


def _get_nc(fast_ln: bool):
    if fast_ln not in _NC_CACHE:
        _NC_CACHE[fast_ln] = build(fast_ln)
    return _NC_CACHE[fast_ln]


def _prep_inputs(inputs):
    h = np.asarray(inputs["h"], np.float32)
    z = np.asarray(inputs["z"], np.float32)
    ln_g = np.asarray(inputs["ln_g"], np.float32)
    ln_b = np.asarray(inputs["ln_b"], np.float32)
    fast_ln = bool(np.all(ln_g == 1.0) and np.all(ln_b == 0.0))

    wt_np = np.concatenate(
        [
            np.asarray(inputs["W_hk"], np.float32),
            np.asarray(inputs["W_hv"], np.float32),
            np.asarray(inputs["W_q"], np.float32),
            np.asarray(inputs["W_zk"], np.float32),
            np.asarray(inputs["W_zv"], np.float32),
        ],
        axis=0,
    ).T  # [512, 6144]
    wt_in = np.ascontiguousarray(wt_np.reshape(DC, 128, W_COLS))
    wout_in = np.ascontiguousarray(
        np.asarray(inputs["W_out"], np.float32).T.reshape(EC, 128, D)
    )
    gb_in = np.ascontiguousarray(
        np.stack(
            [np.broadcast_to(ln_g, (128, D)), np.broadcast_to(ln_b, (128, D))]
        )
    )
    # [core, d-chunk, 128, tokens] feature-major activations
    hT_all = np.ascontiguousarray(
        h.reshape(N_CORES, TPC, D).transpose(0, 2, 1).reshape(N_CORES, DC, 128, TPC)
    )
    zT_all = np.ascontiguousarray(
        z.reshape(N_CORES, TPC, D).transpose(0, 2, 1).reshape(N_CORES, DC, 128, TPC)
    )
    in_maps = [
        {
            "hT": hT_all[i],
            "zT": zT_all[i],
            "wt": wt_in,
            "wout": wout_in,
            "gb": gb_in,
        }
        for i in range(N_CORES)
    ]
    return fast_ln, in_maps


def run(inputs, **spmd_kwargs):
    fast_ln, in_maps = _prep_inputs(inputs)
    nc = _get_nc(fast_ln)
    res = run_bass_kernel_spmd(
        nc, in_maps, core_ids=list(range(N_CORES)), **spmd_kwargs
    )
    outs = np.stack([r["out"] for r in res.results])  # [8, 2048, 512]
    return outs.reshape(N_SEQ, SEQ_K, D).astype(np.float32, copy=False), res


def kernel(**inputs) -> np.ndarray:
    out, _ = run(inputs)
    return out


# revision 26
# speedup vs baseline: 1.0651x; 1.0651x over previous
"""TRN2 Bass kernel for nn_AttentionalDynamicsUpdate (dense transformer block).

Math per sequence (K=64 tokens, D=512, E=2048):
    q = h @ W_q.T; k = [h @ W_hk.T | z @ W_zk.T]; v = [h @ W_hv.T | z @ W_zv.T]
    logits = k @ q.T / sqrt(D); p = softmax(logits, axis=q)
    out = layernorm((p @ v) @ W_out.T)

Kernel reorderings (exact up to fp rounding):
  * (p @ v) @ W_out.T == p @ (v @ W_out.T)  -- turns the 2048-wide attn
    output into a 512-wide "u" computed as one dense batched matmul.
  * layernorm is scale-invariant per row, so the softmax 1/sum(exp)
    normalization is skipped entirely (absorbed by the layernorm).

Data-parallel over the N=256 sequences across 8 cores (32 seqs / core).
Matmul operands are float32r (TF32-like; 1 cycle/row at free-dim >= 256),
accumulation fp32 in PSUM, softmax/layernorm in fp32.
Host pre-transposes h/z/weights so every operand arrives feature-major.
"""

import math

import numpy as np

import concourse.bacc as bacc
import concourse.bass as bass
import concourse.mybir as mybir
import concourse.tile as tile
from concourse.bass_utils import run_bass_kernel_spmd

N_CORES = 8
N_SEQ, SEQ_K, D = 256, 64, 512
E = 2048  # concat feature width (also query width)
TPC = (N_SEQ // N_CORES) * SEQ_K  # tokens per core = 2048
TC = 512  # tokens per pipeline chunk (8 seqs, 4 pairs)
NCH = TPC // TC  # 4 chunks
EC = E // 128  # 16 e-chunks
DC = D // 128  # 4 d-chunks
NPAIR = TC // 128  # 4 seq-pairs per chunk
SCALE = 1.0 / math.sqrt(D)
LN_EPS = 1e-5

F32 = mybir.dt.float32
F32R = mybir.dt.float32r
BF16 = mybir.dt.bfloat16
AX = mybir.AxisListType.X
OP = mybir.AluOpType
AF = mybir.ActivationFunctionType

# wt feature-column layout: [hk 0:1024 | hv 1024:2048 | q 2048:4096 |
#                            zk 4096:5120 | zv 5120:6144]
W_COLS = 6144


def _qkv_src(c):
    """(q_col0, k_src, k_col0, v_src, v_col0) for e-chunk c; src 0=hT 1=zT."""
    q0 = 2048 + 128 * c
    if c < 8:
        return q0, 0, 128 * c, 0, 1024 + 128 * c
    return q0, 1, 4096 + 128 * (c - 8), 1, 5120 + 128 * (c - 8)


def build(fast_ln: bool):
    nc = bacc.Bacc("TRN2", target_bir_lowering=False)

    hT = nc.dram_tensor("hT", [DC, 128, TPC], F32R, kind="ExternalInput")
    zT = nc.dram_tensor("zT", [DC, 128, TPC], F32R, kind="ExternalInput")
    wt = nc.dram_tensor("wt", [DC, 128, W_COLS], F32R, kind="ExternalInput")
    wout = nc.dram_tensor("wout", [EC, 128, D], F32R, kind="ExternalInput")
    gb = nc.dram_tensor("gb", [2, 128, D], F32, kind="ExternalInput")
    ident_dram = nc.inline_tensor(np.eye(128, dtype=np.float32), name="ident128")
    out = nc.dram_tensor("out", [TPC, D], F32, kind="ExternalOutput")

    with tile.TileContext(nc) as tc:
        with (
            tc.tile_pool(name="wpool", bufs=1) as wpool,
            tc.tile_pool(name="xpool", bufs=2) as xpool,
            tc.tile_pool(name="wopool", bufs=3) as wopool,
            tc.tile_pool(name="qkv", bufs=4) as qkv,
            tc.tile_pool(name="attn", bufs=2) as attn,
            tc.tile_pool(name="vecs", bufs=4) as vecs,
            tc.tile_pool(name="psproj", bufs=3, space="PSUM") as psproj,
            tc.tile_pool(name="psu", bufs=4, space="PSUM") as psu,
            tc.tile_pool(name="pslg", bufs=1, space="PSUM") as pslg,
        ):
            # chunk-0 activations first: the first projection matmuls need
            # these plus only the first weight block
            x_tiles = {}
            for tch0 in range(2):
                hT_sb0 = xpool.tile(
                    [128, DC, TC], F32R, name="hT_sb", tag="hT_sb"
                )
                zT_sb0 = xpool.tile(
                    [128, DC, TC], F32R, name="zT_sb", tag="zT_sb"
                )
                x_tiles[tch0] = (hT_sb0, zT_sb0)
            wt_sb = wpool.tile([128, DC, W_COLS], F32R)
            # interleave chunk-0 h with the first-needed weight block so the
            # first projection matmul's inputs land within a few us; z-side
            # loads are deferred (first needed at e-chunk 8)
            for d in range(DC):
                nc.sync.dma_start(x_tiles[0][0][:, d, :], hT[d, :, 0:TC])
                nc.gpsimd.dma_start(wt_sb[:, d, 2048:3072], wt[d, :, 2048:3072])
            for c0, c1 in ((0, 1024), (1024, 2048)):  # hk, hv
                for d in range(DC):
                    eng = nc.gpsimd if d % 2 else nc.sync
                    eng.dma_start(wt_sb[:, d, c0:c1], wt[d, :, c0:c1])
            for d in range(DC):
                nc.gpsimd.dma_start(x_tiles[0][1][:, d, :], zT[d, :, 0:TC])
            for c0, c1 in ((4096, 5120), (5120, 6144), (3072, 4096)):
                for d in range(DC):
                    eng = nc.gpsimd if d % 2 else nc.sync
                    eng.dma_start(wt_sb[:, d, c0:c1], wt[d, :, c0:c1])
            ident = wpool.tile([128, 128], F32)
            nc.sync.dma_start(ident[:], ident_dram[:])
            eps_t = wpool.tile([128, 1], F32)
            nc.vector.memset(eps_t[:], LN_EPS)
            if not fast_ln:
                gtile = wpool.tile([128, D], F32)
                btile = wpool.tile([128, D], F32)
                nc.sync.dma_start(gtile[:], gb[0])
                nc.sync.dma_start(btile[:], gb[1])

            for tch in range(NCH):
                t0 = tch * TC
                if tch == 0:
                    hT_sb, zT_sb = x_tiles[0]
                else:
                    if tch == 1:
                        hT_sb, zT_sb = x_tiles[1]
                    else:
                        hT_sb = xpool.tile(
                            [128, DC, TC], F32R, name="hT_sb", tag="hT_sb"
                        )
                        zT_sb = xpool.tile(
                            [128, DC, TC], F32R, name="zT_sb", tag="zT_sb"
                        )
                    for d in range(DC):
                        nc.sync.dma_start(hT_sb[:, d, :], hT[d, :, t0 : t0 + TC])
                        nc.sync.dma_start(zT_sb[:, d, :], zT[d, :, t0 : t0 + TC])
                srcs = (hT_sb, zT_sb)

                lg_acc = attn.tile(
                    [128, NPAIR, 128], F32, name="lg_acc", tag="lgacc"
                )
                u_ps = [
                    psu.tile([128, D], F32, name=f"u_ps{g}", tag="ub")
                    for g in range(NPAIR)
                ]

                for c in range(EC):
                    q0, ksrc, k0, vsrc, v0 = _qkv_src(c)
                    wo_sb = wopool.tile([128, D], F32R, name="wo_sb", tag="wo")
                    nc.sync.dma_start(wo_sb[:], wout[c])

                    q_ps = psproj.tile([128, TC], F32, name="q_ps", tag="proj")
                    k_ps = psproj.tile([128, TC], F32, name="k_ps", tag="proj")
                    v_ps = psproj.tile([128, TC], F32, name="v_ps", tag="proj")
                    for ps, src, col0 in (
                        (q_ps, hT_sb, q0),
                        (k_ps, srcs[ksrc], k0),
                        (v_ps, srcs[vsrc], v0),
                    ):
                        for d in range(DC):
                            nc.tensor.matmul(
                                ps[:],
                                wt_sb[:, d, col0 : col0 + 128],
                                src[:, d, :],
                                start=(d == 0),
                                stop=(d == DC - 1),
                            )
                    q_sb = qkv.tile([128, TC], BF16, name="q_sb", tag="q_sb")
                    k_sb = qkv.tile([128, TC], BF16, name="k_sb", tag="k_sb")
                    v_sb = qkv.tile([128, TC], F32R, name="v_sb", tag="v_sb")
                    nc.scalar.copy(q_sb[:], q_ps[:])
                    nc.vector.tensor_copy(k_sb[:], k_ps[:])
                    nc.scalar.copy(v_sb[:], v_ps[:])

                    first = c == 0
                    # full [kA|kB] x [qA|qB] blocks; the diagonal 64x64
                    # sub-blocks are the two sequences' logits (cross terms
                    # are discarded). PSUM accumulation across e-chunks is
                    # unsafe (pairs share a bank and start=True clears the
                    # whole bank's has_written bits), so the partials land
                    # in one bank and accumulate in SBUF with one DVE add.
                    lgp = pslg.tile([128, NPAIR, 128], F32, name="lgp", tag="lgp")
                    for p in range(NPAIR):
                        pb = p * 128
                        nc.tensor.matmul(
                            lgp[:, p, :],
                            k_sb[:, pb : pb + 128],
                            q_sb[:, pb : pb + 128],
                        )
                        nc.tensor.matmul(
                            u_ps[p][:],
                            v_sb[:, pb : pb + 128],
                            wo_sb[:],
                            start=first,
                            stop=(c == EC - 1),
                        )
                    if first:
                        nc.vector.tensor_copy(lg_acc[:], lgp[:])
                    else:
                        nc.vector.tensor_add(lg_acc[:], lg_acc[:], lgp[:])

                # phase A: softmax + probs^T + u re-layout for all pairs
                # (groups ACT functions to avoid act-table thrash)
                pt_sbs, u_sbs = [], []
                for p in range(NPAIR):
                    mx = vecs.tile([128, 1], F32, name="mx", tag="mx")
                    mneg = vecs.tile([128, 1], F32, name="mneg", tag="mneg")
                    nc.vector.reduce_max(mx[0:64], lg_acc[0:64, p, 0:64], axis=AX)
                    nc.vector.reduce_max(
                        mx[64:128], lg_acc[64:128, p, 64:128], axis=AX
                    )
                    nc.vector.tensor_scalar_mul(mneg[:], mx[:], -SCALE)
                    probs = attn.tile([128, 64], F32, name="probs", tag="probs")
                    nc.scalar.activation(
                        probs[0:64, :],
                        lg_acc[0:64, p, 0:64],
                        AF.Exp,
                        bias=mneg[0:64],
                        scale=SCALE,
                    )
                    nc.scalar.activation(
                        probs[64:128, :],
                        lg_acc[64:128, p, 64:128],
                        AF.Exp,
                        bias=mneg[64:128],
                        scale=SCALE,
                    )
                    # probs^T: [64 q, 128 (kA|kB)]; transpose out must sit at
                    # PSUM partition 0
                    pt_ps = pslg.tile([64, 128], F32, name="pt_ps", tag="lgp")
                    nc.tensor.transpose(pt_ps[:], probs[:], ident[:])
                    pt_sb = attn.tile(
                        [64, 128], F32R, name=f"pt_sb{p}", tag=f"ptsb{p}", bufs=1
                    )
                    nc.vector.tensor_copy(pt_sb[:], pt_ps[:])
                    pt_sbs.append(pt_sb)

                    # u re-layout: both seq halves to partition base 0 (DMA
                    # shifts partitions; DVE cannot; DMA cannot read PSUM)
                    u_st = attn.tile([128, D], F32R, name="u_st", tag="ust")
                    nc.scalar.copy(u_st[:], u_ps[p][:])
                    u_sb = attn.tile(
                        [64, 2, D], F32R, name=f"u_sb{p}", tag=f"usb{p}", bufs=1
                    )
                    nc.vector.tensor_copy(u_sb[:, 0, :], u_st[0:64, :])
                    nc.sync.dma_start(u_sb[:, 1, :], u_st[64:128, :])
                    u_sbs.append(u_sb)

                # phase B: out matmuls + layernorm + store
                for p in range(NPAIR):
                    pt_sb, u_sb = pt_sbs[p], u_sbs[p]
                    o_ps = [
                        psu.tile([64, D], F32, name=f"o_ps{si}", tag="ub")
                        for si in range(2)
                    ]
                    nc.tensor.matmul(o_ps[0][:], pt_sb[:, 0:64], u_sb[:, 0, :])
                    nc.tensor.matmul(o_ps[1][:], pt_sb[:, 64:128], u_sb[:, 1, :])

                    # layernorm over D; softmax 1/sum already absorbed here
                    for si in range(2):
                        oraw = attn.tile([64, D], F32, name="oraw", tag="oraw")
                        nc.vector.tensor_copy(oraw[:], o_ps[si][:])
                        ops = oraw
                        sm = vecs.tile([64, 1], F32, name="sm", tag="sm")
                        ssq = vecs.tile([64, 1], F32, name="ssq", tag="ssq")
                        mu = vecs.tile([64, 1], F32, name="mu", tag="mu")
                        mu2 = vecs.tile([64, 1], F32, name="mu2", tag="mu2")
                        var = vecs.tile([64, 1], F32, name="var", tag="var")
                        sd = vecs.tile([64, 1], F32, name="sd", tag="sd")
                        rstd = vecs.tile([64, 1], F32, name="rstd", tag="rstd")
                        c1 = vecs.tile([64, 1], F32, name="c1", tag="c1")
                        scr = attn.tile([64, D], F32, name="scr", tag="scr")
                        nc.vector.reduce_sum(sm[:], ops[:], axis=AX)
                        nc.scalar.activation(
                            scr[:], ops[:], AF.Square, accum_out=ssq[:]
                        )
                        nc.vector.tensor_scalar_mul(mu[:], sm[:], 1.0 / D)
                        nc.vector.tensor_mul(mu2[:], mu[:], mu[:])
                        nc.vector.tensor_scalar_mul(var[:], ssq[:], 1.0 / D)
                        nc.vector.tensor_sub(var[:], var[:], mu2[:])
                        nc.scalar.activation(sd[:], var[:], AF.Sqrt, bias=eps_t[0:64])
                        nc.vector.reciprocal(rstd[:], sd[:])
                        nc.vector.tensor_scalar(
                            c1[:], mu[:], rstd[:], -1.0, op0=OP.mult, op1=OP.mult
                        )
                        o_sb = attn.tile([64, D], F32, name="o_sb", tag="osb")
                        nc.vector.tensor_scalar(
                            o_sb[:], ops[:], rstd[:], c1[:], op0=OP.mult, op1=OP.add
                        )
                        if not fast_ln:
                            nc.vector.tensor_mul(o_sb[:], o_sb[:], gtile[0:64, :])
                            nc.vector.tensor_add(o_sb[:], o_sb[:], btile[0:64, :])
                        r0 = t0 + p * 128 + si * 64
                        nc.sync.dma_start(out[r0 : r0 + 64, :], o_sb[:])

    nc.compile()
    return nc


_NC_CACHE = {}


def _get_nc(fast_ln: bool):
    if fast_ln not in _NC_CACHE:
        _NC_CACHE[fast_ln] = build(fast_ln)
    return _NC_CACHE[fast_ln]


def _prep_inputs(inputs):
    h = np.asarray(inputs["h"], np.float32)
    z = np.asarray(inputs["z"], np.float32)
    ln_g = np.asarray(inputs["ln_g"], np.float32)
    ln_b = np.asarray(inputs["ln_b"], np.float32)
    fast_ln = bool(np.all(ln_g == 1.0) and np.all(ln_b == 0.0))

    wt_np = np.concatenate(
        [
            np.asarray(inputs["W_hk"], np.float32),
            np.asarray(inputs["W_hv"], np.float32),
            np.asarray(inputs["W_q"], np.float32),
            np.asarray(inputs["W_zk"], np.float32),
            np.asarray(inputs["W_zv"], np.float32),
        ],
        axis=0,
    ).T  # [512, 6144]
    wt_in = np.ascontiguousarray(wt_np.reshape(DC, 128, W_COLS))
    wout_in = np.ascontiguousarray(
        np.asarray(inputs["W_out"], np.float32).T.reshape(EC, 128, D)
    )
    gb_in = np.ascontiguousarray(
        np.stack(
            [np.broadcast_to(ln_g, (128, D)), np.broadcast_to(ln_b, (128, D))]
        )
    )
    # [core, d-chunk, 128, tokens] feature-major activations
    hT_all = np.ascontiguousarray(
        h.reshape(N_CORES, TPC, D).transpose(0, 2, 1).reshape(N_CORES, DC, 128, TPC)
    )
    zT_all = np.ascontiguousarray(
        z.reshape(N_CORES, TPC, D).transpose(0, 2, 1).reshape(N_CORES, DC, 128, TPC)
    )
    in_maps = [
        {
            "hT": hT_all[i],
            "zT": zT_all[i],
            "wt": wt_in,
            "wout": wout_in,
            "gb": gb_in,
        }
        for i in range(N_CORES)
    ]
    return fast_ln, in_maps


def run(inputs, **spmd_kwargs):
    fast_ln, in_maps = _prep_inputs(inputs)
    nc = _get_nc(fast_ln)
    res = run_bass_kernel_spmd(
        nc, in_maps, core_ids=list(range(N_CORES)), **spmd_kwargs
    )
    outs = np.stack([r["out"] for r in res.results])  # [8, 2048, 512]
    return outs.reshape(N_SEQ, SEQ_K, D).astype(np.float32, copy=False), res


def kernel(**inputs) -> np.ndarray:
    out, _ = run(inputs)
    return out
